# revision 7
# baseline (speedup 1.0000x reference)
"""GAT layer kernel for Trainium2, data-parallel over batch across 8 NeuronCores.

Per batch element b (one core each):
    hp  = h @ W_proj + b_proj                      # [N, D]
    s   = hp @ w_src ; t = hp @ w_dst              # [N]
    e   = relu(s[:,None] + t[None,:] + b_att)      # [N, N]
    att = exp(e) * a ; att /= att.sum(-1, keepdim) # [N, N]
    out = att @ hp + hp                            # [N, D]

Key identity: exp(relu(x)) == max(exp(x), 1), so the relu disappears into a
tensor_scalar_max and the exp runs directly on ACT with per-partition bias.

Per 128-row block of the score matrix:
  - SWDGE cast-DMA loads a-block f32->bf16
  - ACT: z = Exp(t_full + bias=s_col)          (bf16 out)
  - DVE: zc = max(z, 1)                        (4x bf16 mode)
  - DVE: P = zc * a, rowsum(P)                 (tensor_tensor_reduce, 2x mode)
  - one batched xbar DMA-transpose P -> P^T    (blocked [128,16,128] layout)
  - PE: 16 bf16 matmuls accumulate P^T.T @ hp into PSUM
  - ACT: out = PSUM * (1/rowsum)  ;  DVE: out += hp
"""

import os
import sys

for _p in ("/opt/trn_rl_repo", "/root/.axon_site/_ro/trn_rl_repo"):
    if _p not in sys.path and os.path.isdir(_p):
        sys.path.append(_p)

import numpy as np
from contextlib import ExitStack

import concourse.bass as bass
import concourse.bacc as bacc
import concourse.tile as tile
from concourse import masks, mybir
from concourse.bass_utils import run_bass_kernel_spmd

F32 = mybir.dt.float32
BF16 = mybir.dt.bfloat16

B, N, D = 8, 2048, 128
P = 128           # partitions
NT = N // P       # 16 row/col blocks
N_CORES = 8


def _build_kernel(ctx: ExitStack, tc: tile.TileContext, io: dict):
    nc = tc.nc
    a = io["a"]            # [N, N] f32 dram
    h = io["h"]            # [N, D] f32 dram
    W = io["W_proj"]       # [D, D] f32 dram
    b_proj = io["b_proj"]  # [D, 1] f32 dram
    w_src = io["w_src"]    # [D, 1] f32 dram
    w_dst = io["w_dst"]    # [D, 1] f32 dram
    b_att = io["b_att"]    # [1, 1] f32 dram
    out = io["out"]        # [N, D] f32 dram

    cst = ctx.enter_context(tc.tile_pool(name="cst", bufs=1))
    sps = ctx.enter_context(tc.tile_pool(name="sps", bufs=3, space="PSUM"))

    # ---- constants / weights ----
    ident = cst.tile([P, P], F32)
    masks.make_identity(nc, ident[:])

    W_sb = cst.tile([P, D], F32)
    nc.sync.dma_start(W_sb[:], W[:])
    bp_col = cst.tile([P, 1], F32)
    nc.sync.dma_start(bp_col[:], b_proj[:])
    ws_col = cst.tile([P, 1], F32)
    nc.sync.dma_start(ws_col[:], w_src[:])
    wd_col = cst.tile([P, 1], F32)
    nc.sync.dma_start(wd_col[:], w_dst[:])
    ba_sb = cst.tile([1, 1], F32)
    nc.sync.dma_start(ba_sb[:], b_att[:])

    # ---- h natural tiles: [p, r, d] with h[r*128+p, d] ----
    h_sb = cst.tile([P, NT, D], F32)
    nc.sync.dma_start(h_sb[:], h.rearrange("(r p) d -> p r d", p=P))

    # ---- hT [in, n] via PE transposes ----
    hT = cst.tile([P, N], F32)
    for r in range(NT):
        ps = sps.tile([P, 512], F32, tag="sps")
        nc.tensor.matmul(ps[:, :P], h_sb[:, r, :], ident[:], is_transpose=True)
        nc.scalar.copy(hT[:, r * P:(r + 1) * P], ps[:, :P])

    # ---- hpT [d, n] = (h @ W + b).T : lhsT=W [in,d], rhs=hT [in,n] ----
    hpT = cst.tile([P, N], F32)
    for s4 in range(4):
        sl = slice(s4 * 512, (s4 + 1) * 512)
        ps = sps.tile([P, 512], F32, tag="sps")
        nc.tensor.matmul(ps[:], W_sb[:], hT[:, sl])
        nc.scalar.activation(hpT[:, sl], ps[:],
                             mybir.ActivationFunctionType.Identity,
                             bias=bp_col[:], scale=1.0)

    # ---- hp natural (f32 for the residual add; bf16 + ones column for the
    # matmul rhs: P @ [hp | 1] puts the row-sum in psum column D) ----
    hp_nat = cst.tile([P, NT, D], F32)
    hp_aug = cst.tile([P, NT, D + 1], BF16)
    nc.vector.memset(hp_aug[:, :, D:D + 1], 1.0)
    for r in range(NT):
        ps = sps.tile([P, 512], F32, tag="sps")
        nc.tensor.matmul(ps[:, :P], hpT[:, r * P:(r + 1) * P], ident[:],
                         is_transpose=True)
        nc.scalar.copy(hp_nat[:, r, :], ps[:, :P])
        nc.vector.tensor_copy(hp_aug[:, r, :D], hp_nat[:, r, :])

    # ---- s_col [p, r]: s[i] = hp[i,:] @ w_src, i = r*128+p ----
    s_col = cst.tile([P, NT], F32)
    s_ps = sps.tile([P, 512], F32, tag="sps")
    for r in range(NT):
        nc.tensor.matmul(s_ps[:, r:r + 1], hpT[:, r * P:(r + 1) * P], ws_col[:])
    nc.scalar.copy(s_col[:], s_ps[:, :NT])

    # ---- t_row [1, n] = hp @ w_dst + b_att; replicate across partitions ----
    t_row = cst.tile([1, N], F32)
    for s4 in range(4):
        sl = slice(s4 * 512, (s4 + 1) * 512)
        ps = sps.tile([1, 512], F32, tag="spst")
        nc.tensor.matmul(ps[:], wd_col[:], hpT[:, sl])
        nc.scalar.activation(t_row[:, sl], ps[:],
                             mybir.ActivationFunctionType.Identity,
                             bias=ba_sb[:], scale=1.0)
    t_full = cst.tile([P, N], F32)
    nc.gpsimd.partition_broadcast(t_full[:], t_row[:])

    # ---- main loop pools ----
    a_pool = ctx.enter_context(tc.tile_pool(name="a", bufs=3))
    z_pool = ctx.enter_context(tc.tile_pool(name="z", bufs=2))
    zc_pool = ctx.enter_context(tc.tile_pool(name="zc", bufs=2))
    pb_pool = ctx.enter_context(tc.tile_pool(name="pb", bufs=2))
    pbt_pool = ctx.enter_context(tc.tile_pool(name="pbt", bufs=2))
    rs_pool = ctx.enter_context(tc.tile_pool(name="rs", bufs=2))
    osb_pool = ctx.enter_context(tc.tile_pool(name="osb", bufs=2))
    ops_pool = ctx.enter_context(tc.tile_pool(name="ops", bufs=2, space="PSUM"))

    out_stage = cst.tile([P, NT, D], F32)

    for r in range(NT):
        a_t = a_pool.tile([P, N], BF16)
        nc.gpsimd.dma_start(a_t[:], a[r * P:(r + 1) * P, :])  # SWDGE f32->bf16

        z_t = z_pool.tile([P, N], BF16)
        nc.scalar.activation(z_t[:], t_full[:],
                             mybir.ActivationFunctionType.Exp,
                             bias=s_col[:, r:r + 1], scale=1.0)

        zc_t = zc_pool.tile([P, N], BF16)
        nc.vector.tensor_scalar_max(zc_t[:], z_t[:], 1.0)

        pb_t = pb_pool.tile([P, N], BF16)
        nc.vector.tensor_mul(pb_t[:], zc_t[:], a_t[:])

        pbT_t = pbt_pool.tile([P, NT, P], BF16)
        nc.sync.dma_start_transpose(out=pbT_t[:], in_=pb_t[:])

        o_ps = ops_pool.tile([P, D + 1], F32)
        for c in range(NT):
            nc.tensor.matmul(o_ps[:], pbT_t[:, c, :], hp_aug[:, c, :],
                             start=(c == 0), stop=(c == NT - 1))

        rinv = rs_pool.tile([P, 1], F32, tag="rinv")
        nc.vector.reciprocal(rinv[:], o_ps[:, D:D + 1])
        o_sb = osb_pool.tile([P, D], F32)
        nc.scalar.activation(o_sb[:], o_ps[:, :D],
                             mybir.ActivationFunctionType.Copy,
                             scale=rinv[:])
        nc.vector.tensor_add(out_stage[:, r, :], o_sb[:], hp_nat[:, r, :])

    nc.sync.dma_start(out.rearrange("(r p) d -> p r d", p=P), out_stage[:])


_CACHE = {}


def _get_compiled():
    if "nc" in _CACHE:
        return _CACHE["nc"], _CACHE["names"]

    nc = bacc.Bacc("TRN2", target_bir_lowering=False, debug=False)
    io = {}
    io["a"] = nc.dram_tensor("a", [N, N], F32, kind="ExternalInput").ap()
    io["h"] = nc.dram_tensor("h", [N, D], F32, kind="ExternalInput").ap()
    io["W_proj"] = nc.dram_tensor("W_proj", [D, D], F32, kind="ExternalInput").ap()
    io["b_proj"] = nc.dram_tensor("b_proj", [D, 1], F32, kind="ExternalInput").ap()
    io["w_src"] = nc.dram_tensor("w_src", [D, 1], F32, kind="ExternalInput").ap()
    io["w_dst"] = nc.dram_tensor("w_dst", [D, 1], F32, kind="ExternalInput").ap()
    io["b_att"] = nc.dram_tensor("b_att", [1, 1], F32, kind="ExternalInput").ap()
    io["out"] = nc.dram_tensor("out", [N, D], F32, kind="ExternalOutput").ap()

    with tile.TileContext(nc) as tc:
        with ExitStack() as ctx:
            _build_kernel(ctx, tc, io)
    nc.compile()

    _CACHE["nc"] = nc
    _CACHE["names"] = list(io.keys())
    return nc, _CACHE["names"]


def _make_in_maps(a, h, W_proj, b_proj, w_att, b_att):
    a = np.ascontiguousarray(a, dtype=np.float32)
    h = np.ascontiguousarray(h, dtype=np.float32)
    W_proj = np.ascontiguousarray(W_proj, dtype=np.float32)
    b_proj = np.ascontiguousarray(b_proj, dtype=np.float32).reshape(D, 1)
    w_att = np.ascontiguousarray(w_att, dtype=np.float32)
    w_src = w_att[:D].reshape(D, 1).copy()
    w_dst = w_att[D:].reshape(D, 1).copy()
    b_att = np.asarray(b_att, dtype=np.float32).reshape(1, 1).copy()

    in_maps = []
    for c in range(N_CORES):
        in_maps.append({
            "a": a[c], "h": h[c], "W_proj": W_proj, "b_proj": b_proj,
            "w_src": w_src, "w_dst": w_dst, "b_att": b_att,
        })
    return in_maps


def _get_executable():
    """Build (once) a sharded PJRT callable for the compiled Bass module.

    Mirrors concourse.bass2jax.run_bass_via_pjrt but keeps the jitted
    function so repeated calls don't retrace/recompile.
    """
    if "exe" in _CACHE:
        return _CACHE["exe"]

    import jax
    from jax.sharding import Mesh, PartitionSpec
    from jax.experimental.shard_map import shard_map
    from concourse import bass2jax, mybir as _mybir

    nc, _ = _get_compiled()
    bass2jax.install_neuronx_cc_hook()

    partition_name = (nc.partition_id_tensor.name
                      if nc.partition_id_tensor else None)
    in_names, out_names, out_avals, zero_outs = [], [], [], []
    for alloc in nc.m.functions[0].allocations:
        if not isinstance(alloc, _mybir.MemoryLocationSet):
            continue
        name = alloc.memorylocations[0].name
        if alloc.kind == "ExternalInput":
            if name != partition_name:
                in_names.append(name)
        elif alloc.kind == "ExternalOutput":
            shape = tuple(alloc.tensor_shape)
            dtype = _mybir.dt.np(alloc.dtype)
            out_names.append(name)
            out_avals.append(jax.core.ShapedArray(shape, dtype))
            zero_outs.append(np.zeros(shape, dtype))
    n_params = len(in_names)
    n_outs = len(out_avals)
    all_in_names = in_names + out_names + (
        [partition_name] if partition_name else [])
    donate = tuple(range(n_params, n_params + n_outs))

    def _body(*args):
        operands = list(args)
        if partition_name is not None:
            operands.append(bass2jax.partition_id_tensor())
        outs = bass2jax._bass_exec_p.bind(
            *operands,
            out_avals=tuple(out_avals),
            in_names=tuple(all_in_names),
            out_names=tuple(out_names),
            lowering_input_output_aliases=(),
            sim_require_finite=True,
            sim_require_nnan=True,
            nc=nc,
        )
        return tuple(outs)

    devices = jax.devices()[:N_CORES]
    mesh = Mesh(np.asarray(devices), ("core",))
    in_specs = (PartitionSpec("core"),) * (n_params + n_outs)
    out_specs = (PartitionSpec("core"),) * n_outs
    fn = jax.jit(
        shard_map(_body, mesh=mesh, in_specs=in_specs, out_specs=out_specs,
                  check_rep=False),
        donate_argnums=donate, keep_unused=True,
    )
    exe = {
        "fn": fn, "mesh": mesh, "in_names": in_names,
        "out_names": out_names, "out_avals": out_avals,
        "zero_outs": zero_outs, "n_params": n_params,
    }
    _CACHE["exe"] = exe
    return exe


def _concat_inputs(exe, in_maps):
    return [
        np.concatenate([np.asarray(in_maps[c][name])
                        for c in range(N_CORES)], axis=0)
        for name in exe["in_names"]
    ]


def _concat_zeros(exe):
    return [np.zeros((N_CORES * z.shape[0], *z.shape[1:]), z.dtype)
            for z in exe["zero_outs"]]


def kernel(a, h, W_proj, b_proj, w_att, b_att):
    exe = _get_executable()
    in_maps = _make_in_maps(a, h, W_proj, b_proj, w_att, b_att)
    out_arrs = exe["fn"](*_concat_inputs(exe, in_maps), *_concat_zeros(exe))
    i = exe["out_names"].index("out")
    return np.asarray(out_arrs[i]).reshape(N_CORES, N, D)


if __name__ == "__main__":
    rng = np.random.default_rng(0)
    a = rng.random((B, N, N), dtype=np.float32)
    h = rng.standard_normal((B, N, D), dtype=np.float32)
    W_proj = (rng.standard_normal((D, D)) / np.sqrt(D)).astype(np.float32)
    b_proj = (rng.standard_normal(D) * 0.01).astype(np.float32)
    w_att = (rng.standard_normal(2 * D) / np.sqrt(2 * D)).astype(np.float32)
    b_att = np.float32(rng.standard_normal() * 0.01)

    got = kernel(a=a, h=h, W_proj=W_proj, b_proj=b_proj, w_att=w_att, b_att=b_att)

    hp = h @ W_proj + b_proj
    s = hp @ w_att[:D]
    t = hp @ w_att[D:]
    e = np.maximum(s[:, :, None] + t[:, None, :] + b_att, 0.0)
    att = np.exp(e) * a
    att = att / att.sum(-1, keepdims=True)
    ref = att @ hp + hp

    err = np.abs(got - ref).max() / np.abs(ref).max()
    print("rel err:", err)


# revision 9
# speedup vs baseline: 7.6663x; 7.6663x over previous
"""GAT layer kernel for Trainium2, data-parallel over batch across 8 NeuronCores.

Per batch element b (one core each):
    hp  = h @ W_proj + b_proj                      # [N, D]
    s   = hp @ w_src ; t = hp @ w_dst              # [N]
    e   = relu(s[:,None] + t[None,:] + b_att)      # [N, N]
    att = exp(e) * a ; att /= att.sum(-1, keepdim) # [N, N]
    out = att @ hp + hp                            # [N, D]

Key identity: exp(relu(x)) == max(exp(x), 1), so the relu disappears into a
tensor_scalar_max and the exp runs directly on ACT with per-partition bias.

Per 128-row block of the score matrix:
  - SWDGE cast-DMA loads a-block f32->bf16
  - ACT: z = Exp(t_full + bias=s_col)          (bf16 out)
  - DVE: zc = max(z, 1)                        (4x bf16 mode)
  - DVE: P = zc * a, rowsum(P)                 (tensor_tensor_reduce, 2x mode)
  - one batched xbar DMA-transpose P -> P^T    (blocked [128,16,128] layout)
  - PE: 16 bf16 matmuls accumulate P^T.T @ hp into PSUM
  - ACT: out = PSUM * (1/rowsum)  ;  DVE: out += hp
"""

import os
import sys

for _p in ("/opt/trn_rl_repo", "/root/.axon_site/_ro/trn_rl_repo"):
    if _p not in sys.path and os.path.isdir(_p):
        sys.path.append(_p)

import numpy as np
from contextlib import ExitStack

import concourse.bass as bass
import concourse.bacc as bacc
import concourse.tile as tile
from concourse import masks, mybir
from concourse.bass_utils import run_bass_kernel_spmd

F32 = mybir.dt.float32
BF16 = mybir.dt.bfloat16

B, N, D = 8, 2048, 128
P = 128           # partitions
NT = N // P       # 16 row/col blocks
N_CORES = 8


def _build_kernel(ctx: ExitStack, tc: tile.TileContext, io: dict):
    nc = tc.nc
    a = io["a"]            # [N, N] f32 dram
    h = io["h"]            # [N, D] f32 dram
    W = io["W_proj"]       # [D, D] f32 dram
    b_proj = io["b_proj"]  # [D, 1] f32 dram
    w_src = io["w_src"]    # [D, 1] f32 dram
    w_dst = io["w_dst"]    # [D, 1] f32 dram
    b_att = io["b_att"]    # [1, 1] f32 dram
    out = io["out"]        # [N, D] f32 dram

    cst = ctx.enter_context(tc.tile_pool(name="cst", bufs=1))
    sps = ctx.enter_context(tc.tile_pool(name="sps", bufs=2, space="PSUM"))

    # ---- constants / weights ----
    ident = cst.tile([P, P], F32)
    masks.make_identity(nc, ident[:])

    W_sb = cst.tile([P, D], F32)
    nc.sync.dma_start(W_sb[:], W[:])
    bp_col = cst.tile([P, 1], F32)
    nc.sync.dma_start(bp_col[:], b_proj[:])
    ws_col = cst.tile([P, 1], F32)
    nc.sync.dma_start(ws_col[:], w_src[:])
    wd_col = cst.tile([P, 1], F32)
    nc.sync.dma_start(wd_col[:], w_dst[:])
    ba_sb = cst.tile([1, 1], F32)
    nc.sync.dma_start(ba_sb[:], b_att[:])

    # ---- h natural tiles: [p, r, d] with h[r*128+p, d] ----
    h_sb = cst.tile([P, NT, D], F32)
    nc.sync.dma_start(h_sb[:], h.rearrange("(r p) d -> p r d", p=P))

    # ---- hT [in, n] via PE transposes ----
    hT = cst.tile([P, N], F32)
    for r in range(NT):
        ps = sps.tile([P, 512], F32, tag="sps")
        nc.tensor.matmul(ps[:, :P], h_sb[:, r, :], ident[:], is_transpose=True)
        nc.scalar.copy(hT[:, r * P:(r + 1) * P], ps[:, :P])

    # ---- hpT [d, n] = (h @ W + b).T : lhsT=W [in,d], rhs=hT [in,n] ----
    hpT = cst.tile([P, N], F32)
    for s4 in range(4):
        sl = slice(s4 * 512, (s4 + 1) * 512)
        ps = sps.tile([P, 512], F32, tag="sps")
        nc.tensor.matmul(ps[:], W_sb[:], hT[:, sl])
        nc.scalar.activation(hpT[:, sl], ps[:],
                             mybir.ActivationFunctionType.Identity,
                             bias=bp_col[:], scale=1.0)

    # ---- hp natural (f32 for the residual add; bf16 + ones column for the
    # matmul rhs: P @ [hp | 1] puts the row-sum in psum column D) ----
    hp_nat = cst.tile([P, NT, D], F32)
    hp_aug = cst.tile([P, NT, D + 1], BF16)
    nc.vector.memset(hp_aug[:, :, D:D + 1], 1.0)
    for r in range(NT):
        ps = sps.tile([P, 512], F32, tag="sps")
        nc.tensor.matmul(ps[:, :P], hpT[:, r * P:(r + 1) * P], ident[:],
                         is_transpose=True)
        nc.scalar.copy(hp_nat[:, r, :], ps[:, :P])
        nc.vector.tensor_copy(hp_aug[:, r, :D], hp_nat[:, r, :])

    # ---- s_col [p, r]: s[i] = hp[i,:] @ w_src, i = r*128+p ----
    s_col = cst.tile([P, NT], F32)
    s_ps = sps.tile([P, 512], F32, tag="sps")
    for r in range(NT):
        nc.tensor.matmul(s_ps[:, r:r + 1], hpT[:, r * P:(r + 1) * P], ws_col[:])
    nc.scalar.copy(s_col[:], s_ps[:, :NT])

    # ---- t_row [1, n] = hp @ w_dst + b_att; replicate across partitions ----
    t_row = cst.tile([1, N], F32)
    for s4 in range(4):
        sl = slice(s4 * 512, (s4 + 1) * 512)
        ps = sps.tile([1, 512], F32, tag="spst")
        nc.tensor.matmul(ps[:], wd_col[:], hpT[:, sl])
        nc.scalar.activation(t_row[:, sl], ps[:],
                             mybir.ActivationFunctionType.Identity,
                             bias=ba_sb[:], scale=1.0)
    t_full = cst.tile([P, N], F32)
    nc.gpsimd.partition_broadcast(t_full[:], t_row[:])

    # ---- main loop pools ----
    a_pool = ctx.enter_context(tc.tile_pool(name="a", bufs=4))
    z_pool = ctx.enter_context(tc.tile_pool(name="z", bufs=3))
    zc_pool = ctx.enter_context(tc.tile_pool(name="zc", bufs=3))
    pb_pool = ctx.enter_context(tc.tile_pool(name="pb", bufs=3))
    pbt_pool = ctx.enter_context(tc.tile_pool(name="pbt", bufs=3))
    rs_pool = ctx.enter_context(tc.tile_pool(name="rs", bufs=4))
    osb_pool = ctx.enter_context(tc.tile_pool(name="osb", bufs=3))
    ops_pool = ctx.enter_context(tc.tile_pool(name="ops", bufs=4, space="PSUM"))

    out_stage = cst.tile([P, NT, D], F32)

    # Finalize (reciprocal/scale/residual) is lagged LAG iterations so the
    # DVE's in-order stream never stalls on the current iteration's matmuls.
    LAG = 2
    pending = []

    def finalize(o_ps, r):
        rinv = rs_pool.tile([P, 1], F32, tag="rinv")
        nc.vector.reciprocal(rinv[:], o_ps[:, D:D + 1])
        o_sb = osb_pool.tile([P, D], F32)
        nc.scalar.activation(o_sb[:], o_ps[:, :D],
                             mybir.ActivationFunctionType.Copy,
                             scale=rinv[:])
        nc.vector.tensor_add(out_stage[:, r, :], o_sb[:], hp_nat[:, r, :])

    for r in range(NT):
        a_t = a_pool.tile([P, N], BF16)
        nc.gpsimd.dma_start(a_t[:], a[r * P:(r + 1) * P, :])  # SWDGE f32->bf16

        z_t = z_pool.tile([P, N], BF16)
        nc.scalar.activation(z_t[:], t_full[:],
                             mybir.ActivationFunctionType.Exp,
                             bias=s_col[:, r:r + 1], scale=1.0)

        zc_t = zc_pool.tile([P, N], BF16)
        nc.vector.tensor_scalar_max(zc_t[:], z_t[:], 1.0)

        pb_t = pb_pool.tile([P, N], BF16)
        nc.vector.tensor_mul(pb_t[:], zc_t[:], a_t[:])

        pbT_t = pbt_pool.tile([P, NT, P], BF16)
        nc.sync.dma_start_transpose(out=pbT_t[:], in_=pb_t[:])

        o_ps = ops_pool.tile([P, D + 1], F32)
        for c in range(NT):
            nc.tensor.matmul(o_ps[:], pbT_t[:, c, :], hp_aug[:, c, :],
                             start=(c == 0), stop=(c == NT - 1))

        pending.append((o_ps, r))
        if len(pending) > LAG:
            finalize(*pending.pop(0))

    for item in pending:
        finalize(*item)

    nc.sync.dma_start(out.rearrange("(r p) d -> p r d", p=P), out_stage[:])


_CACHE = {}


def _get_compiled():
    if "nc" in _CACHE:
        return _CACHE["nc"], _CACHE["names"]

    nc = bacc.Bacc("TRN2", target_bir_lowering=False, debug=False)
    io = {}
    io["a"] = nc.dram_tensor("a", [N, N], F32, kind="ExternalInput").ap()
    io["h"] = nc.dram_tensor("h", [N, D], F32, kind="ExternalInput").ap()
    io["W_proj"] = nc.dram_tensor("W_proj", [D, D], F32, kind="ExternalInput").ap()
    io["b_proj"] = nc.dram_tensor("b_proj", [D, 1], F32, kind="ExternalInput").ap()
    io["w_src"] = nc.dram_tensor("w_src", [D, 1], F32, kind="ExternalInput").ap()
    io["w_dst"] = nc.dram_tensor("w_dst", [D, 1], F32, kind="ExternalInput").ap()
    io["b_att"] = nc.dram_tensor("b_att", [1, 1], F32, kind="ExternalInput").ap()
    io["out"] = nc.dram_tensor("out", [N, D], F32, kind="ExternalOutput").ap()

    with tile.TileContext(nc) as tc:
        with ExitStack() as ctx:
            _build_kernel(ctx, tc, io)
    nc.compile()

    _CACHE["nc"] = nc
    _CACHE["names"] = list(io.keys())
    return nc, _CACHE["names"]


def _make_in_maps(a, h, W_proj, b_proj, w_att, b_att):
    a = np.ascontiguousarray(a, dtype=np.float32)
    h = np.ascontiguousarray(h, dtype=np.float32)
    W_proj = np.ascontiguousarray(W_proj, dtype=np.float32)
    b_proj = np.ascontiguousarray(b_proj, dtype=np.float32).reshape(D, 1)
    w_att = np.ascontiguousarray(w_att, dtype=np.float32)
    w_src = w_att[:D].reshape(D, 1).copy()
    w_dst = w_att[D:].reshape(D, 1).copy()
    b_att = np.asarray(b_att, dtype=np.float32).reshape(1, 1).copy()

    in_maps = []
    for c in range(N_CORES):
        in_maps.append({
            "a": a[c], "h": h[c], "W_proj": W_proj, "b_proj": b_proj,
            "w_src": w_src, "w_dst": w_dst, "b_att": b_att,
        })
    return in_maps


def _get_executable():
    """Build (once) a sharded PJRT callable for the compiled Bass module.

    Mirrors concourse.bass2jax.run_bass_via_pjrt but keeps the jitted
    function so repeated calls don't retrace/recompile.
    """
    if "exe" in _CACHE:
        return _CACHE["exe"]

    import jax
    from jax.sharding import Mesh, PartitionSpec
    from jax.experimental.shard_map import shard_map
    from concourse import bass2jax, mybir as _mybir

    nc, _ = _get_compiled()
    bass2jax.install_neuronx_cc_hook()

    partition_name = (nc.partition_id_tensor.name
                      if nc.partition_id_tensor else None)
    in_names, out_names, out_avals, zero_outs = [], [], [], []
    for alloc in nc.m.functions[0].allocations:
        if not isinstance(alloc, _mybir.MemoryLocationSet):
            continue
        name = alloc.memorylocations[0].name
        if alloc.kind == "ExternalInput":
            if name != partition_name:
                in_names.append(name)
        elif alloc.kind == "ExternalOutput":
            shape = tuple(alloc.tensor_shape)
            dtype = _mybir.dt.np(alloc.dtype)
            out_names.append(name)
            out_avals.append(jax.core.ShapedArray(shape, dtype))
            zero_outs.append(np.zeros(shape, dtype))
    n_params = len(in_names)
    n_outs = len(out_avals)
    all_in_names = in_names + out_names + (
        [partition_name] if partition_name else [])
    donate = tuple(range(n_params, n_params + n_outs))

    def _body(*args):
        operands = list(args)
        if partition_name is not None:
            operands.append(bass2jax.partition_id_tensor())
        outs = bass2jax._bass_exec_p.bind(
            *operands,
            out_avals=tuple(out_avals),
            in_names=tuple(all_in_names),
            out_names=tuple(out_names),
            lowering_input_output_aliases=(),
            sim_require_finite=True,
            sim_require_nnan=True,
            nc=nc,
        )
        return tuple(outs)

    devices = jax.devices()[:N_CORES]
    mesh = Mesh(np.asarray(devices), ("core",))
    in_specs = (PartitionSpec("core"),) * (n_params + n_outs)
    out_specs = (PartitionSpec("core"),) * n_outs
    fn = jax.jit(
        shard_map(_body, mesh=mesh, in_specs=in_specs, out_specs=out_specs,
                  check_rep=False),
        donate_argnums=donate, keep_unused=True,
    )
    exe = {
        "fn": fn, "mesh": mesh, "in_names": in_names,
        "out_names": out_names, "out_avals": out_avals,
        "zero_outs": zero_outs, "n_params": n_params,
    }
    _CACHE["exe"] = exe
    return exe


def _concat_inputs(exe, in_maps):
    return [
        np.concatenate([np.asarray(in_maps[c][name])
                        for c in range(N_CORES)], axis=0)
        for name in exe["in_names"]
    ]


def _concat_zeros(exe):
    return [np.zeros((N_CORES * z.shape[0], *z.shape[1:]), z.dtype)
            for z in exe["zero_outs"]]


def kernel(a, h, W_proj, b_proj, w_att, b_att):
    exe = _get_executable()
    in_maps = _make_in_maps(a, h, W_proj, b_proj, w_att, b_att)
    out_arrs = exe["fn"](*_concat_inputs(exe, in_maps), *_concat_zeros(exe))
    i = exe["out_names"].index("out")
    return np.asarray(out_arrs[i]).reshape(N_CORES, N, D)


if __name__ == "__main__":
    rng = np.random.default_rng(0)
    a = rng.random((B, N, N), dtype=np.float32)
    h = rng.standard_normal((B, N, D), dtype=np.float32)
    W_proj = (rng.standard_normal((D, D)) / np.sqrt(D)).astype(np.float32)
    b_proj = (rng.standard_normal(D) * 0.01).astype(np.float32)
    w_att = (rng.standard_normal(2 * D) / np.sqrt(2 * D)).astype(np.float32)
    b_att = np.float32(rng.standard_normal() * 0.01)

    got = kernel(a=a, h=h, W_proj=W_proj, b_proj=b_proj, w_att=w_att, b_att=b_att)

    hp = h @ W_proj + b_proj
    s = hp @ w_att[:D]
    t = hp @ w_att[D:]
    e = np.maximum(s[:, :, None] + t[:, None, :] + b_att, 0.0)
    att = np.exp(e) * a
    att = att / att.sum(-1, keepdims=True)
    ref = att @ hp + hp

    err = np.abs(got - ref).max() / np.abs(ref).max()
    print("rel err:", err)
